# revision 16
# baseline (speedup 1.0000x reference)
"""Trainium2 Bass kernel for ContrastiveMSELoss.

Reference computes, over all N^2 pairs (diagonal masked to 0):
    mse_ij  = (|x_i|^2 + |x_j|^2 - 2 x_i.x_j) / D
    sign_ij = +1 if class_i == class_j else -1
    loss    = mean_ij(sign_ij * mse_ij) + BETA

Using sum_{i,j in c} x_i.x_j = |M_c|^2 with M_c = sum_{i in c} x_i, the
loss collapses to class-bucketed first/second moments (O(N*D) work,
memory-bound -- no N x N gram matrix needed):

    T_same = sum_c (2 n_c SQ_c - 2 |M_c|^2) / D      (diag terms are 0)
    T_all  = (2 N SQ - 2 |M|^2) / D
    loss   = (2 T_same - T_all) / N^2 + BETA

Sharding: rows are split across 8 cores (1024 rows each).  Per core the
shard maps row r = p*8 + k onto partition p, sub-chunk k, so every DMA
block is contiguous per partition (2 KB descriptors, per-SDMA-engine
line rate).  x blocks are spread over the two HWDGE rings.  A
one-hot-classes bf16 matmul accumulates per-class sums of X and X^2 in
PSUM (separate 256-wide matmuls so the X side never waits on the
squares), with even sub-chunks targeting PSUM partitions 0:40 and odd
ones 64:104 so consecutive matmuls use different PE column groups and
run concurrently.  The result store runs after the TileContext, and
nothing waits on its HBM write receipt -- the NEFF epilogue's DMA
drain covers it.  Host combines the per-core [104, 512] partials
(rows 40:64 are padding).
"""

import numpy as np

import concourse.bacc as bacc
import concourse.bass as bass
import concourse.tile as tile
from concourse import mybir
from concourse.bass_utils import run_bass_kernel_spmd

N, D = 8192, 256
N_CORES = 8
ROWS = N // N_CORES          # 1024 rows per core
P = 128                      # partitions
K = ROWS // P                # 8 sub-rows per partition (row = p*K + k)
NCLS = 40
BETA = 1.0
OUTP = 104                   # output partitions: rows 0:40 + padding + 64:104

_CACHE = {}


def _bcast(ap, pos, count):
    """Insert a zero-stride dim of size `count` at free-dim position `pos`."""
    pattern = [list(p) for p in ap.ap]
    pattern.insert(pos, [0, count])
    return bass.AP(tensor=ap.tensor, offset=ap.offset, ap=pattern)


def _build_bass():
    nc = bacc.Bacc(
        "TRN2",
        target_bir_lowering=False,
        debug=False,
        enable_asserts=False,
        num_devices=N_CORES,
    )
    # x shard viewed as [128, 8, 256]: partition p = rows p*8 .. p*8+7
    x = nc.dram_tensor("x", [P, K, D], mybir.dt.float32, kind="ExternalInput")
    # combo[p, :NCLS] = iota row 0..39; combo[p, NCLS + k] = class of row p*8+k
    combo = nc.dram_tensor(
        "combo", [P, NCLS + K], mybir.dt.float32, kind="ExternalInput"
    )
    # stats rows 0:40 = even-k chain, 64:104 = odd-k chain, 40:64 garbage;
    # cols 0:256 per-class sums of x, 256:512 per-class sums of x^2
    stats = nc.dram_tensor(
        "stats", [OUTP, 2 * D], mybir.dt.bfloat16, kind="ExternalOutput"
    )

    sem_out = nc.alloc_semaphore("out_dma")
    # raw (non-Tile) SBUF tensor so the post-TileContext store has a
    # concrete access pattern
    out_sb = nc.alloc_sbuf_tensor("out_sb_raw", [P, 2 * D], mybir.dt.bfloat16)

    with tile.TileContext(nc) as tc:
        with (
            tc.tile_pool(name="work", bufs=1) as work,
            tc.tile_pool(name="psum", bufs=1, space="PSUM") as psum_pool,
        ):
            xf = work.tile([P, K, D], mybir.dt.float32, tag="xf")
            xbx = work.tile([P, K, D], mybir.dt.bfloat16, tag="xbx")
            xbs = work.tile([P, K, D], mybir.dt.bfloat16, tag="xbs")
            combo_sb = work.tile([P, NCLS + K], mybir.dt.float32, tag="combo_sb")
            oh = work.tile([P, K, NCLS], mybir.dt.bfloat16, tag="oh")
            acc = psum_pool.tile([P, 2 * D], mybir.dt.float32, tag="acc")

            # Input DMAs.  Sync ring: combo + blocks 0/1; scalar ring
            # (starts ~1us later in practice): blocks 2/3.
            nc.sync.dma_start(out=combo_sb, in_=combo[:, :])
            nc.sync.dma_start(out=xf[:, 0:2, :], in_=x[:, 0:2, :])
            nc.scalar.dma_start(out=xf[:, 4:6, :], in_=x[:, 4:6, :])
            nc.sync.dma_start(out=xf[:, 2:4, :], in_=x[:, 2:4, :])
            nc.scalar.dma_start(out=xf[:, 6:8, :], in_=x[:, 6:8, :])

            iota_sb = combo_sb[:, :NCLS]
            cls_sb = combo_sb[:, NCLS:]

            # one-hot: oh[p, k, c] = (cls[p, k] == c)
            nc.vector.tensor_tensor(
                out=oh[:, :, :],
                in0=_bcast(cls_sb, 2, NCLS),
                in1=_bcast(iota_sb, 1, K),
                op=mybir.AluOpType.is_equal,
            )

            for b in range(4):
                k0 = 2 * b
                # cast X -> bf16 (DVE), then the X matmul pair right away
                nc.vector.tensor_copy(
                    xbx[:, k0 : k0 + 2, :], xf[:, k0 : k0 + 2, :]
                )
                for k in (k0, k0 + 1):
                    lo = 0 if k % 2 == 0 else 64
                    nc.tensor.matmul(
                        acc[lo : lo + NCLS, :D],
                        oh[:, k, :],
                        xbx[:, k, :],
                        start=(k < 2),
                        stop=(k >= K - 2),
                        skip_group_check=True,
                    )
                # squares: ACT (from f32, independent of the cast) for the
                # first three blocks, DVE (bf16 2x from the cast) for the
                # last so neither engine paces the tail
                if b < 3:
                    nc.scalar.activation(
                        out=xbs[:, k0 : k0 + 2, :],
                        in_=xf[:, k0 : k0 + 2, :],
                        func=mybir.ActivationFunctionType.Square,
                    )
                else:
                    nc.vector.tensor_mul(
                        xbs[:, k0 : k0 + 2, :],
                        xbx[:, k0 : k0 + 2, :],
                        xbx[:, k0 : k0 + 2, :],
                    )
                for k in (k0, k0 + 1):
                    lo = 0 if k % 2 == 0 else 64
                    nc.tensor.matmul(
                        acc[lo : lo + NCLS, D:],
                        oh[:, k, :],
                        xbs[:, k, :],
                        start=(k < 2),
                        stop=(k >= K - 2),
                        skip_group_check=True,
                    )

            # fold both chains' PSUM slices to the raw SBUF tensor (bf16) on
            # two engines in parallel; still inside the TileContext so the
            # PSUM reads are ordered after the matmuls
            nc.vector.tensor_copy(out_sb[:NCLS, :], acc[:NCLS, :])
            nc.scalar.copy(out_sb[64 : 64 + NCLS, :], acc[64 : 64 + NCLS, :])

    # TileContext exit emits an all-engine barrier, so the store below is
    # ordered after the folds by plain program order.  Deliberately nothing
    # waits on sem_out: the NEFF epilogue drains the DMA rings before
    # execution completes, which guarantees the store has landed by the
    # time the host reads `stats`.
    nc.sync.dma_start(out=stats[:, :], in_=out_sb[:OUTP, :]).then_inc(sem_out, 16)

    return nc


def _get_nc():
    if "nc" not in _CACHE:
        nc = _build_bass()
        nc.finalize()
        _CACHE["nc"] = nc
    return _CACHE["nc"]


_IOTA = np.broadcast_to(np.arange(NCLS, dtype=np.float32), (P, NCLS))


def run_device(output, classes, **spmd_kwargs):
    """Run the per-core Bass kernel; returns (list of per-core stats, results)."""
    x = np.ascontiguousarray(np.asarray(output), dtype=np.float32)
    cls_f = np.asarray(classes).astype(np.float32)
    in_maps = []
    for s in range(N_CORES):
        xs = x[s * ROWS : (s + 1) * ROWS].reshape(P, K, D)
        cs = cls_f[s * ROWS : (s + 1) * ROWS].reshape(P, K)
        combo = np.concatenate([_IOTA, cs], axis=1)
        in_maps.append(
            {"x": np.ascontiguousarray(xs), "combo": np.ascontiguousarray(combo)}
        )
    res = run_bass_kernel_spmd(
        _get_nc(), in_maps, core_ids=list(range(N_CORES)), **spmd_kwargs
    )
    stats = [res.results[s]["stats"] for s in range(N_CORES)]
    return stats, res


def _combine(stats, classes):
    """Combine per-core partial class stats into the scalar loss (float64)."""
    tot = np.sum(np.asarray(stats, dtype=np.float64), axis=0)  # [104, 512]
    tot = tot[:NCLS] + tot[64 : 64 + NCLS]                     # [40, 512]
    M_c = tot[:, :D]                                           # class sums
    SQ_c = tot[:, D:].sum(axis=1)                              # class |x|^2 sums
    n_c = np.bincount(np.asarray(classes).astype(np.int64), minlength=NCLS).astype(
        np.float64
    )
    SQ = SQ_c.sum()
    M = M_c.sum(axis=0)
    T_same = (2.0 * (n_c * SQ_c).sum() - 2.0 * (M_c * M_c).sum()) / D
    T_all = (2.0 * N * SQ - 2.0 * (M @ M)) / D
    loss = (2.0 * T_same - T_all) / (float(N) * float(N)) + BETA
    return np.float32(loss)


def kernel(output, classes):
    stats, _ = run_device(output, classes)
    return _combine(stats, classes)


# revision 17
# speedup vs baseline: 1.0974x; 1.0974x over previous
"""Trainium2 Bass kernel for ContrastiveMSELoss.

Reference computes, over all N^2 pairs (diagonal masked to 0):
    mse_ij  = (|x_i|^2 + |x_j|^2 - 2 x_i.x_j) / D
    sign_ij = +1 if class_i == class_j else -1
    loss    = mean_ij(sign_ij * mse_ij) + BETA

Using sum_{i,j in c} x_i.x_j = |M_c|^2 with M_c = sum_{i in c} x_i, the
loss collapses to class-bucketed first/second moments (O(N*D) work,
memory-bound -- no N x N gram matrix needed):

    T_same = sum_c (2 n_c SQ_c - 2 |M_c|^2) / D      (diag terms are 0)
    T_all  = (2 N SQ - 2 |M|^2) / D
    loss   = (2 T_same - T_all) / N^2 + BETA

Sharding: rows are split across 8 cores (1024 rows each).  Per core the
shard maps row r = p*8 + k onto partition p, sub-chunk k, so every DMA
block is contiguous per partition (2 KB descriptors, per-SDMA-engine
line rate).  The host pre-builds the one-hot class matrix (from the
tiny classes input) and ships it as a bf16 tensor; x blocks are spread
over the two HWDGE rings.  A one-hot matmul accumulates per-class sums
of X and X^2 in PSUM (separate 256-wide matmuls so the X side never
waits on the squares), with even sub-chunks targeting PSUM partitions
0:40 and odd ones 64:104 so consecutive matmuls use different PE
column groups and run concurrently.  One DVE copy folds PSUM 0:104 to
SBUF, and the result store runs after the TileContext with nothing
waiting on its HBM write receipt -- the NEFF epilogue's DMA drain
covers it.  Host combines the per-core [104, 512] partials (rows 40:64
are padding).
"""

import numpy as np

import concourse.bacc as bacc
import concourse.bass as bass
import concourse.tile as tile
from concourse import mybir
from concourse.bass_utils import run_bass_kernel_spmd

N, D = 8192, 256
N_CORES = 8
ROWS = N // N_CORES          # 1024 rows per core
P = 128                      # partitions
K = ROWS // P                # 8 sub-rows per partition (row = p*K + k)
NCLS = 40
BETA = 1.0
OUTP = 104                   # output partitions: rows 0:40 + padding + 64:104

_CACHE = {}


def _build_bass():
    nc = bacc.Bacc(
        "TRN2",
        target_bir_lowering=False,
        debug=False,
        enable_asserts=False,
        num_devices=N_CORES,
    )
    # x shard viewed as [128, 8, 256]: partition p = rows p*8 .. p*8+7
    x = nc.dram_tensor("x", [P, K, D], mybir.dt.float32, kind="ExternalInput")
    # host-built one-hot: ohd[p, k, c] = (class[p*8+k] == c)
    ohd = nc.dram_tensor(
        "oh", [P, K, NCLS], mybir.dt.bfloat16, kind="ExternalInput"
    )
    # stats rows 0:40 = even-k chain, 64:104 = odd-k chain, 40:64 garbage;
    # cols 0:256 per-class sums of x, 256:512 per-class sums of x^2
    stats = nc.dram_tensor(
        "stats", [OUTP, 2 * D], mybir.dt.bfloat16, kind="ExternalOutput"
    )

    sem_out = nc.alloc_semaphore("out_dma")
    # raw (non-Tile) SBUF tensor so the post-TileContext store has a
    # concrete access pattern
    out_sb = nc.alloc_sbuf_tensor("out_sb_raw", [P, 2 * D], mybir.dt.bfloat16)

    with tile.TileContext(nc) as tc:
        with (
            tc.tile_pool(name="work", bufs=1) as work,
            tc.tile_pool(name="psum", bufs=1, space="PSUM") as psum_pool,
        ):
            xf = work.tile([P, K, D], mybir.dt.float32, tag="xf")
            xbx = work.tile([P, K, D], mybir.dt.bfloat16, tag="xbx")
            xbs = work.tile([P, K, D], mybir.dt.bfloat16, tag="xbs")
            oh = work.tile([P, K, NCLS], mybir.dt.bfloat16, tag="oh")
            acc = psum_pool.tile([P, 2 * D], mybir.dt.float32, tag="acc")

            # Input DMAs.  Sync ring: one-hot + blocks 0/1; scalar ring
            # (starts ~1us later in practice): blocks 2/3.
            nc.sync.dma_start(out=oh[:, :, :], in_=ohd[:, :, :])
            nc.sync.dma_start(out=xf[:, 0:2, :], in_=x[:, 0:2, :])
            nc.scalar.dma_start(out=xf[:, 4:6, :], in_=x[:, 4:6, :])
            nc.sync.dma_start(out=xf[:, 2:4, :], in_=x[:, 2:4, :])
            nc.scalar.dma_start(out=xf[:, 6:8, :], in_=x[:, 6:8, :])

            for b in range(4):
                k0 = 2 * b
                # cast X -> bf16 (DVE), then the X matmul pair right away
                nc.vector.tensor_copy(
                    xbx[:, k0 : k0 + 2, :], xf[:, k0 : k0 + 2, :]
                )
                for k in (k0, k0 + 1):
                    lo = 0 if k % 2 == 0 else 64
                    nc.tensor.matmul(
                        acc[lo : lo + NCLS, :D],
                        oh[:, k, :],
                        xbx[:, k, :],
                        start=(k < 2),
                        stop=(k >= K - 2),
                        skip_group_check=True,
                    )
                # squares: ACT (from f32, independent of the cast) for the
                # first three blocks, DVE (bf16 2x from the cast) for the
                # last so neither engine paces the tail
                if b < 3:
                    nc.scalar.activation(
                        out=xbs[:, k0 : k0 + 2, :],
                        in_=xf[:, k0 : k0 + 2, :],
                        func=mybir.ActivationFunctionType.Square,
                    )
                else:
                    nc.vector.tensor_mul(
                        xbs[:, k0 : k0 + 2, :],
                        xbx[:, k0 : k0 + 2, :],
                        xbx[:, k0 : k0 + 2, :],
                    )
                for k in (k0, k0 + 1):
                    lo = 0 if k % 2 == 0 else 64
                    nc.tensor.matmul(
                        acc[lo : lo + NCLS, D:],
                        oh[:, k, :],
                        xbs[:, k, :],
                        start=(k < 2),
                        stop=(k >= K - 2),
                        skip_group_check=True,
                    )

            # single fold: PSUM partitions 0:104 -> SBUF bf16 in one DVE op
            # (partition count does not change DVE time; 40:64 is garbage)
            nc.vector.tensor_copy(out_sb[:OUTP, :], acc[:OUTP, :])

    # TileContext exit emits an all-engine barrier, so the store below is
    # ordered after the fold by plain program order.  Deliberately nothing
    # waits on sem_out: the NEFF epilogue drains the DMA rings before
    # execution completes, which guarantees the store has landed by the
    # time the host reads `stats`.
    nc.sync.dma_start(out=stats[:, :], in_=out_sb[:OUTP, :]).then_inc(sem_out, 16)

    return nc


def _get_nc():
    if "nc" not in _CACHE:
        nc = _build_bass()
        nc.finalize()
        _CACHE["nc"] = nc
    return _CACHE["nc"]


def run_device(output, classes, **spmd_kwargs):
    """Run the per-core Bass kernel; returns (list of per-core stats, results)."""
    x = np.ascontiguousarray(np.asarray(output), dtype=np.float32)
    cls = np.asarray(classes).astype(np.int64)
    onehot = (cls[:, None] == np.arange(NCLS)[None, :]).astype(np.float32)
    from ml_dtypes import bfloat16

    onehot = onehot.astype(bfloat16)
    in_maps = []
    for s in range(N_CORES):
        xs = x[s * ROWS : (s + 1) * ROWS].reshape(P, K, D)
        ohs = onehot[s * ROWS : (s + 1) * ROWS].reshape(P, K, NCLS)
        in_maps.append(
            {"x": np.ascontiguousarray(xs), "oh": np.ascontiguousarray(ohs)}
        )
    res = run_bass_kernel_spmd(
        _get_nc(), in_maps, core_ids=list(range(N_CORES)), **spmd_kwargs
    )
    stats = [res.results[s]["stats"] for s in range(N_CORES)]
    return stats, res


def _combine(stats, classes):
    """Combine per-core partial class stats into the scalar loss (float64)."""
    tot = np.sum(np.asarray(stats, dtype=np.float64), axis=0)  # [104, 512]
    tot = tot[:NCLS] + tot[64 : 64 + NCLS]                     # [40, 512]
    M_c = tot[:, :D]                                           # class sums
    SQ_c = tot[:, D:].sum(axis=1)                              # class |x|^2 sums
    n_c = np.bincount(np.asarray(classes).astype(np.int64), minlength=NCLS).astype(
        np.float64
    )
    SQ = SQ_c.sum()
    M = M_c.sum(axis=0)
    T_same = (2.0 * (n_c * SQ_c).sum() - 2.0 * (M_c * M_c).sum()) / D
    T_all = (2.0 * N * SQ - 2.0 * (M @ M)) / D
    loss = (2.0 * T_same - T_all) / (float(N) * float(N)) + BETA
    return np.float32(loss)


def kernel(output, classes):
    stats, _ = run_device(output, classes)
    return _combine(stats, classes)


# revision 18
# speedup vs baseline: 1.1215x; 1.0219x over previous
"""Trainium2 Bass kernel for ContrastiveMSELoss.

Reference computes, over all N^2 pairs (diagonal masked to 0):
    mse_ij  = (|x_i|^2 + |x_j|^2 - 2 x_i.x_j) / D
    sign_ij = +1 if class_i == class_j else -1
    loss    = mean_ij(sign_ij * mse_ij) + BETA

Using sum_{i,j in c} x_i.x_j = |M_c|^2 with M_c = sum_{i in c} x_i, the
loss collapses to class-bucketed first/second moments (O(N*D) work,
memory-bound -- no N x N gram matrix needed):

    T_same = sum_c (2 n_c SQ_c - 2 |M_c|^2) / D      (diag terms are 0)
    T_all  = (2 N SQ - 2 |M|^2) / D
    loss   = (2 T_same - T_all) / N^2 + BETA

Sharding: rows are split across 8 cores (1024 rows each).  Per core the
shard maps row r = p*8 + k onto partition p, sub-chunk k, so every DMA
block is contiguous per partition (2 KB descriptors, per-SDMA-engine
line rate).  The host pre-builds the one-hot class matrix (from the
tiny classes input) and ships it as a bf16 tensor; x blocks are spread
over the two HWDGE rings.  A one-hot matmul accumulates per-class sums
of X and X^2 in PSUM (separate 256-wide matmuls so the X side never
waits on the squares), with even sub-chunks targeting PSUM partitions
0:40 and odd ones 64:104 so consecutive matmuls use different PE
column groups and run concurrently.  One DVE copy folds PSUM 0:104 to
SBUF, and the result store runs after the TileContext with nothing
waiting on its HBM write receipt -- the NEFF epilogue's DMA drain
covers it.  Host combines the per-core [104, 512] partials (rows 40:64
are padding).
"""

import numpy as np

import concourse.bacc as bacc
import concourse.bass as bass
import concourse.tile as tile
from concourse import mybir
from concourse.bass_utils import run_bass_kernel_spmd

N, D = 8192, 256
N_CORES = 8
ROWS = N // N_CORES          # 1024 rows per core
P = 128                      # partitions
K = ROWS // P                # 8 sub-rows per partition (row = p*K + k)
NCLS = 40
BETA = 1.0
OUTP = 104                   # output partitions: rows 0:40 + padding + 64:104

_CACHE = {}


def _build_bass():
    nc = bacc.Bacc(
        "TRN2",
        target_bir_lowering=False,
        debug=False,
        enable_asserts=False,
        num_devices=N_CORES,
    )
    # x shard viewed as [128, 8, 256]: partition p = rows p*8 .. p*8+7
    x = nc.dram_tensor("x", [P, K, D], mybir.dt.float32, kind="ExternalInput")
    # host-built one-hot: ohd[p, k, c] = (class[p*8+k] == c)
    ohd = nc.dram_tensor(
        "oh", [P, K, NCLS], mybir.dt.bfloat16, kind="ExternalInput"
    )
    # stats rows 0:40 = even-k chain, 64:104 = odd-k chain, 40:64 garbage;
    # cols 0:256 per-class sums of x, 256:512 per-class sums of x^2
    stats = nc.dram_tensor(
        "stats", [OUTP, 2 * D], mybir.dt.bfloat16, kind="ExternalOutput"
    )

    sem_out = nc.alloc_semaphore("out_dma")
    # raw (non-Tile) SBUF tensor so the post-TileContext store has a
    # concrete access pattern
    out_sb = nc.alloc_sbuf_tensor("out_sb_raw", [P, 2 * D], mybir.dt.bfloat16)

    with tile.TileContext(nc) as tc:
        with (
            tc.tile_pool(name="work", bufs=1) as work,
            tc.tile_pool(name="psum", bufs=1, space="PSUM") as psum_pool,
        ):
            xbx = work.tile([P, K, D], mybir.dt.bfloat16, tag="xbx")
            xbs = work.tile([P, K, D], mybir.dt.bfloat16, tag="xbs")
            oh = work.tile([P, K, NCLS], mybir.dt.bfloat16, tag="oh")
            acc = psum_pool.tile([P, 2 * D], mybir.dt.float32, tag="acc")

            # Input DMAs.  x blocks ride the gpsimd SWDGE ring, casting
            # f32 -> bf16 in flight (HBM read traffic unchanged), which
            # removes the on-chip cast pass entirely.  The one-hot rides
            # the sync HWDGE ring.
            nc.sync.dma_start(out=oh[:, :, :], in_=ohd[:, :, :])
            for b in range(4):
                k0 = 2 * b
                nc.gpsimd.dma_start(
                    out=xbx[:, k0 : k0 + 2, :], in_=x[:, k0 : k0 + 2, :]
                )

            for b in range(4):
                k0 = 2 * b
                for k in (k0, k0 + 1):
                    lo = 0 if k % 2 == 0 else 64
                    nc.tensor.matmul(
                        acc[lo : lo + NCLS, :D],
                        oh[:, k, :],
                        xbx[:, k, :],
                        start=(k < 2),
                        stop=(k >= K - 2),
                        skip_group_check=True,
                    )
                # squares from the bf16 blocks: ACT for the first two
                # blocks, DVE (bf16 2x) for the last two
                if b < 2:
                    nc.scalar.activation(
                        out=xbs[:, k0 : k0 + 2, :],
                        in_=xbx[:, k0 : k0 + 2, :],
                        func=mybir.ActivationFunctionType.Square,
                    )
                else:
                    nc.vector.tensor_mul(
                        xbs[:, k0 : k0 + 2, :],
                        xbx[:, k0 : k0 + 2, :],
                        xbx[:, k0 : k0 + 2, :],
                    )
                for k in (k0, k0 + 1):
                    lo = 0 if k % 2 == 0 else 64
                    nc.tensor.matmul(
                        acc[lo : lo + NCLS, D:],
                        oh[:, k, :],
                        xbs[:, k, :],
                        start=(k < 2),
                        stop=(k >= K - 2),
                        skip_group_check=True,
                    )

            # single fold: PSUM partitions 0:104 -> SBUF bf16 in one DVE op
            # (partition count does not change DVE time; 40:64 is garbage)
            nc.vector.tensor_copy(out_sb[:OUTP, :], acc[:OUTP, :])

    # TileContext exit emits an all-engine barrier, so the store below is
    # ordered after the fold by plain program order.  Deliberately nothing
    # waits on sem_out: the NEFF epilogue drains the DMA rings before
    # execution completes, which guarantees the store has landed by the
    # time the host reads `stats`.
    nc.sync.dma_start(out=stats[:, :], in_=out_sb[:OUTP, :]).then_inc(sem_out, 16)

    return nc


def _get_nc():
    if "nc" not in _CACHE:
        nc = _build_bass()
        nc.finalize()
        _CACHE["nc"] = nc
    return _CACHE["nc"]


def run_device(output, classes, **spmd_kwargs):
    """Run the per-core Bass kernel; returns (list of per-core stats, results)."""
    x = np.ascontiguousarray(np.asarray(output), dtype=np.float32)
    cls = np.asarray(classes).astype(np.int64)
    onehot = (cls[:, None] == np.arange(NCLS)[None, :]).astype(np.float32)
    from ml_dtypes import bfloat16

    onehot = onehot.astype(bfloat16)
    in_maps = []
    for s in range(N_CORES):
        xs = x[s * ROWS : (s + 1) * ROWS].reshape(P, K, D)
        ohs = onehot[s * ROWS : (s + 1) * ROWS].reshape(P, K, NCLS)
        in_maps.append(
            {"x": np.ascontiguousarray(xs), "oh": np.ascontiguousarray(ohs)}
        )
    res = run_bass_kernel_spmd(
        _get_nc(), in_maps, core_ids=list(range(N_CORES)), **spmd_kwargs
    )
    stats = [res.results[s]["stats"] for s in range(N_CORES)]
    return stats, res


def _combine(stats, classes):
    """Combine per-core partial class stats into the scalar loss (float64)."""
    tot = np.sum(np.asarray(stats, dtype=np.float64), axis=0)  # [104, 512]
    tot = tot[:NCLS] + tot[64 : 64 + NCLS]                     # [40, 512]
    M_c = tot[:, :D]                                           # class sums
    SQ_c = tot[:, D:].sum(axis=1)                              # class |x|^2 sums
    n_c = np.bincount(np.asarray(classes).astype(np.int64), minlength=NCLS).astype(
        np.float64
    )
    SQ = SQ_c.sum()
    M = M_c.sum(axis=0)
    T_same = (2.0 * (n_c * SQ_c).sum() - 2.0 * (M_c * M_c).sum()) / D
    T_all = (2.0 * N * SQ - 2.0 * (M @ M)) / D
    loss = (2.0 * T_same - T_all) / (float(N) * float(N)) + BETA
    return np.float32(loss)


def kernel(output, classes):
    stats, _ = run_device(output, classes)
    return _combine(stats, classes)
